# revision 8
# baseline (speedup 1.0000x reference)
"""CollaborativeAttention Trainium2 kernel (v2).

Reference computation (B=16, S=512, D=512, H=8, DK=DV=DO=512, TB=64):
    q = x @ Wq.T ; k = x @ Wk.T
    mixed_q[b,h,s,i] = q[b,s,i] * mixing[h,i]
    scores = mixed_q @ k.T + tbias(T)[:,None] + cb.T[:, :, None, :]
    scores = mask(scores) / sqrt(64); probs = softmax(scores)
    v = (x @ Wv.T + bv) split into 8 heads of 64
    ctx = probs @ v ; out = ctx @ Wd.T + bd ; y = LayerNorm(x + out)

Key structure (v2, evolved from the v1 T-major kernel):
  * tbias MLP collapsed to an affine form on the host; the multiplicative
    softmax factor eb = exp(tbias/8)*mask01 is fully precomputed on the host
    (bf16) -- no Ln/Exp chain on the device scalar engine.
  * content bias cb = x @ Wcb.T / 8 precomputed on host (0.1% of FLOPs).
  * q/k/v projections and the output dense run as fp8e4 DoubleRow matmuls
    (2x PE throughput); weights are host-upscaled by SW=32 so fp8 stays in
    its normal range, and the scale is folded into the exp and the residual.
  * scores stay bf16 (making the per-head mixed-q fp8 would cost more DVE
    time than the PE saves).
  * softmax denominator fused into the ctx matmul: the stationary operand is
    [v_h | ones] via a strided AP over one [128, KB, 9, 64] tile, so one
    matmul yields ctx rows 0:64 and the denominator in rows 64:128.
  * normalization 2/den via Ln/Exp on ACT with the ctx8 fp8 upscale (x64)
    folded into the Exp bias.

Layout: T-major everywhere ([t, s] / [d, s]); queries rotated by PERM so the
causal mask becomes right-aligned column ranges; no device transposes.

Sharding: pure data-parallel over batch, 2 batches per core, 8 cores.
"""

import math

import numpy as np

import ml_dtypes

import concourse.bass as bass
import concourse.mybir as mybir
import concourse.tile as tile
from concourse.bass_utils import run_bass_kernel_spmd

# ------------------------------------------------------------------ constants
B, S, D = 16, 512, 512
H = 8
DK = DV = DO = 512
TB = 64
EH = DV // H  # 64, per-head value dim
N_CORES = 8
BPC = B // N_CORES  # batches per core
KB = D // 128  # 4 k-blocks of 128
LN_EPS = 1e-5

F32 = mybir.dt.float32
BF16 = mybir.dt.bfloat16
FP8 = mybir.dt.float8e4

SW = 32.0  # fp8 weight upscale
SCX = 64.0  # ctx8 upscale
SCD = SW * SW * SCX / SW  # net dense psum scale = SW * SCX = 2048
EXP_SCALE = 1.0 / (SW * SW)  # undo qk fp8 weight scales inside the exp

DR = mybir.MatmulPerfMode.DoubleRow

CFG = {"mm": "bf16", "pt_engine": "vector"}


def _fp8(a):
    return np.clip(np.asarray(a, np.float32), -240.0, 240.0).astype(
        ml_dtypes.float8_e4m3fn
    )


# ---------------------------------------------------------------- wait fixup
def _split_multi_waits(nc):
    """This walrus build allows 1 sync wait per instruction (2 on
    EventSemaphore).  Tile's final drain carries one wait per live semaphore;
    split the excess into preceding EventSemaphore instructions."""
    counter = 0
    for fn in nc.m.functions:
        for bb in fn.blocks:
            insts = bb.instructions
            i = 0
            while i < len(insts):
                inst = insts[i]
                si = inst.sync_info
                waits = list(si.on_wait) if si is not None else []
                cap = 2 if isinstance(inst, mybir.InstEventSemaphore) else 1
                if len(waits) > cap:
                    extra, keep = waits[:-cap], waits[-cap:]
                    new_evs = []
                    for j in range(0, len(extra), 2):
                        counter += 1
                        ev = mybir.InstEventSemaphore(
                            name=f"I-waitfix-{counter}",
                            engine=inst.engine,
                            ins=[],
                            outs=[],
                            sync_info=mybir.SyncInfo(
                                on_wait=extra[j : j + 2], on_update=[]
                            ),
                        )
                        nc.register_instruction(ev)
                        new_evs.append(ev)
                    inst.sync_info = mybir.SyncInfo(
                        on_wait=keep, on_update=list(si.on_update)
                    )
                    for k, ev in enumerate(new_evs):
                        insts.insert(i + k, ev)
                    i += len(new_evs)
                i += 1


# ---------------------------------------------------------------- host prep
def _tb_affine(tb1_w, tb1_b, tb2_w, tb2_b, u_min, u_max):
    """Collapse the temporal-bias MLP to tbias = A*u + B over u in
    [u_min, u_max].  Returns (A, B) or None if any leaky-relu breakpoint falls
    strictly inside the range (then the affine form is invalid)."""
    w1 = np.asarray(tb1_w, np.float64).reshape(-1)  # [TB]
    b1 = np.asarray(tb1_b, np.float64).reshape(-1)  # [TB]
    w2 = np.asarray(tb2_w, np.float64).reshape(-1)  # [TB]
    b2 = float(np.asarray(tb2_b, np.float64).reshape(-1)[0])
    lo = w1 * u_min + b1
    hi = w1 * u_max + b1
    if np.any((lo < 0) & (hi > 0)) or np.any((lo > 0) & (hi < 0)):
        return None
    pos = (lo + hi) > 0  # sign of the argument over the whole range
    f = np.where(pos, 1.0, 0.2)
    A = float(np.sum(w2 * f * w1))
    Bc = float(np.sum(w2 * f * b1) + b2)
    return A, Bc


def _prepare(inputs):
    x = np.asarray(inputs["x"], np.float32)
    T = np.asarray(inputs["batch_temporal_mat"], np.float32)
    Wq = np.asarray(inputs["Wq"], np.float32)
    Wk = np.asarray(inputs["Wk"], np.float32)
    Wcb = np.asarray(inputs["Wcb"], np.float32)
    Wv = np.asarray(inputs["Wv"], np.float32)
    bv = np.asarray(inputs["bv"], np.float32)
    mixing = np.asarray(inputs["mixing"], np.float32)
    Wd = np.asarray(inputs["Wd"], np.float32)
    bd = np.asarray(inputs["bd"], np.float32)
    ln_g = np.asarray(inputs["ln_g"], np.float32)
    ln_b = np.asarray(inputs["ln_b"], np.float32)

    inv_sqrt_hs = 1.0 / math.sqrt(DK / H)  # 1/8

    # multiplicative causal mask in [t, s] layout; row0 (s=0) fully visible.
    # The s axis is rotated (s=0 moved to the end) so that for key-block t the
    # needed query columns [128t, 512) + {s=0} become one contiguous range --
    # scores/exp/ctx then run on right-aligned column slices only.
    PERM = np.concatenate([np.arange(1, S), [0]])
    t_idx = np.arange(S)[:, None]
    s_idx = np.arange(S)[None, :]
    m01 = np.where((t_idx > s_idx) & (s_idx != 0), 0.0, 1.0)  # [t, s]
    m01 = m01[:, PERM]

    flags = {
        "bv_zero": not np.any(bv),
        "bd_zero": not np.any(bd),
        "ln_identity": bool(np.all(ln_g == 1.0) and not np.any(ln_b)),
    }

    # eb = exp(tbias/8) * mask01 in rotated [t, s] layout, bf16, on host.
    L = np.log(np.e + T.astype(np.float64))
    u = 1.0 / L  # [B, S(s), S(t)]
    ab = _tb_affine(
        inputs["tb1_w"], inputs["tb1_b"], inputs["tb2_w"], inputs["tb2_b"],
        float(u.min()), float(u.max()),
    )
    if ab is not None:
        A, Bc = ab
        tb = A * u + Bc  # [s, t]
    else:
        w1 = np.asarray(inputs["tb1_w"], np.float64).reshape(-1)
        b1 = np.asarray(inputs["tb1_b"], np.float64).reshape(-1)
        w2 = np.asarray(inputs["tb2_w"], np.float64).reshape(-1)
        b2 = float(np.asarray(inputs["tb2_b"], np.float64).reshape(-1)[0])
        tb = np.empty_like(u)
        for bi in range(u.shape[0]):  # chunk to bound the [S,S,TB] temporary
            hh = u[bi][..., None] * w1 + b1
            hh = np.where(hh > 0, hh, 0.2 * hh)
            tb[bi] = hh @ w2 + b2
    ebts = np.exp(tb * inv_sqrt_hs).transpose(0, 2, 1)[:, :, PERM] * m01
    eb_full = np.ascontiguousarray(ebts).astype(ml_dtypes.bfloat16)

    # content bias on host: cb[b, t, h] / 8 -> [b, 128, KB*H] (col = i*H + h)
    cbv = (x @ (Wcb.T * inv_sqrt_hs)).astype(np.float32)  # [B, S, H]
    cb_full = np.ascontiguousarray(
        cbv.reshape(B, KB, 128, H).transpose(0, 2, 1, 3).reshape(B, 128, KB * H)
    )

    xT = np.ascontiguousarray(x.transpose(0, 2, 1))  # [B, d, t] natural
    xTq = np.ascontiguousarray(xT[:, :, PERM])  # [B, d, s'] query-rotated

    common = {
        "wq8": _fp8(Wq.T * SW),  # [d, i]
        "wk8": _fp8(Wk.T * SW),  # [d, i]
        "wv8": _fp8(Wv.T * SW),  # [d, j]
        "wd8": _fp8(Wd.T * SW),  # [j, o]
        "mixt": np.ascontiguousarray(mixing.T * inv_sqrt_hs).astype(
            np.float32
        ),  # [i, h]
        "consts": np.broadcast_to(
            np.array([np.log(2.0), LN_EPS], np.float32), (128, 2)
        ).copy(),
    }
    if not flags["bv_zero"]:
        common["bvrow"] = (bv * SW).reshape(1, DV).astype(ml_dtypes.bfloat16)
        common["onesrow"] = np.ones((1, 128), ml_dtypes.bfloat16)
    if not flags["bd_zero"]:
        common["bdrow"] = (bd * SCD).reshape(1, DO).astype(ml_dtypes.bfloat16)
        if "onesrow" not in common:
            common["onesrow"] = np.ones((1, 128), ml_dtypes.bfloat16)
    if not flags["ln_identity"]:
        common["lng"] = np.broadcast_to(ln_g, (128, DV)).astype(np.float32).copy()
        common["lnb"] = np.broadcast_to(ln_b, (128, DV)).astype(np.float32).copy()

    in_maps = []
    for c in range(N_CORES):
        sl = slice(c * BPC, (c + 1) * BPC)
        m = dict(common)
        m["xt8"] = _fp8(xT[sl])
        m["xtq8"] = _fp8(xTq[sl])
        m["xr"] = np.ascontiguousarray(x[sl][:, PERM, :]).astype(ml_dtypes.bfloat16)
        m["eb"] = np.ascontiguousarray(eb_full[sl])
        m["cb"] = np.ascontiguousarray(cb_full[sl])
        in_maps.append(m)
    return in_maps, flags


# -------------------------------------------------------------- device build
def build_nc(flags):
    nc = bass.Bass()

    xt8_d = nc.dram_tensor("xt8", [BPC, D, S], FP8, kind="ExternalInput")
    xtq8_d = nc.dram_tensor("xtq8", [BPC, D, S], FP8, kind="ExternalInput")
    xr_d = nc.dram_tensor("xr", [BPC, S, D], BF16, kind="ExternalInput")
    eb_d = nc.dram_tensor("eb", [BPC, S, S], BF16, kind="ExternalInput")
    cb_d = nc.dram_tensor("cb", [BPC, 128, KB * H], F32, kind="ExternalInput")
    wq8_d = nc.dram_tensor("wq8", [D, DK], FP8, kind="ExternalInput")
    wk8_d = nc.dram_tensor("wk8", [D, DK], FP8, kind="ExternalInput")
    wv8_d = nc.dram_tensor("wv8", [D, DV], FP8, kind="ExternalInput")
    wd8_d = nc.dram_tensor("wd8", [DV, DO], FP8, kind="ExternalInput")
    mixt_d = nc.dram_tensor("mixt", [DK, H], F32, kind="ExternalInput")
    consts_d = nc.dram_tensor("consts", [128, 2], F32, kind="ExternalInput")
    if not flags["bv_zero"]:
        bvrow_d = nc.dram_tensor("bvrow", [1, DV], BF16, kind="ExternalInput")
    if not flags["bd_zero"]:
        bdrow_d = nc.dram_tensor("bdrow", [1, DO], BF16, kind="ExternalInput")
    if not flags["bv_zero"] or not flags["bd_zero"]:
        onesrow_d = nc.dram_tensor("onesrow", [1, 128], BF16, kind="ExternalInput")
    if not flags["ln_identity"]:
        lng_d = nc.dram_tensor("lng", [128, DV], F32, kind="ExternalInput")
        lnb_d = nc.dram_tensor("lnb", [128, DV], F32, kind="ExternalInput")
    y_d = nc.dram_tensor("y", [BPC, S, DO], BF16, kind="ExternalOutput")

    mul = mybir.AluOpType.mult
    sub = mybir.AluOpType.subtract
    add = mybir.AluOpType.add
    AF = mybir.ActivationFunctionType

    from contextlib import ExitStack

    with tile.TileContext(nc) as tc:
        with ExitStack() as est:
            pool = lambda name, bufs, **kw: est.enter_context(
                tc.tile_pool(name=name, bufs=bufs, **kw)
            )
            wts = pool("wts", 1)
            xt_p = pool("xt", 2)
            xr_p = pool("xr", 2)
            eb_p = pool("eb", 2)
            cb_p = pool("cb", 2)
            qt_p = pool("qtp", 8)
            qkv_p = pool("qkv", 2)
            mq_p = pool("mq", 6)
            ptx_p = pool("ptx", 4)
            pt_p = pool("pt", 12)
            rs_p = pool("rs", 4)
            ctx_p = pool("ctx", 2)
            vc_p = pool("vc", 3)
            ysb_p = pool("ysb", 3)
            scr_p = pool("scr", 2)
            yout_p = pool("yout", 4)
            st_p = pool("st", 24)
            # psAD: shared by projections (sequential groups) and the dense
            # stage; 4 bufs let the last batch hold 4 open dense groups while
            # heads 4-7 still use psS/psCU (4 + 2 + 2 = 8 banks).
            psAD = pool("psAD", 4, space="PSUM")
            psS = pool("psS", 2, space="PSUM")
            psCU = pool("psCU", 2, space="PSUM")

            # ---- DMA helpers.  Each dma_start is one serial ~0.65us trigger
            # on its issuing engine's queue and lands on one HWDGE queue
            # (round-robin), so: critical data first, few triggers for late
            # data, non-critical triggers on the (idle) gpsimd queue.
            def dma_chunk(dst, src3, c, eng=None):
                (eng or nc.sync).dma_start(
                    dst[:, c : c + 1, :],
                    src3[c * 128 : (c + 1) * 128, :].rearrange(
                        "(k p) n -> p k n", p=128
                    ),
                )

            def dma_split_k(dst, src3, nchunks=KB, eng=None):
                # src3: [D, N] dram; dst: [128, KB, N] tile; chunk = k-block
                per = KB // nchunks
                for c in range(nchunks):
                    (eng or nc.sync).dma_start(
                        dst[:, c * per : (c + 1) * per, :],
                        src3[
                            c * per * 128 : (c + 1) * per * 128, :
                        ].rearrange("(k p) n -> p k n", p=128),
                    )

            wq8 = wts.tile([128, KB, DK], FP8, tag="wq8")
            wk8 = wts.tile([128, KB, DK], FP8, tag="wk8")
            wv8 = wts.tile([128, KB, DV], FP8, tag="wv8")
            wd8 = wts.tile([128, KB, DO], FP8, tag="wd8")
            mixt = wts.tile([128, KB, H], F32, tag="mixt")
            consts = wts.tile([128, 2], F32, tag="consts")
            if not flags["bv_zero"]:
                bvrow = wts.tile([1, DV], BF16, tag="bvrow")
            if not flags["bd_zero"]:
                bdrow = wts.tile([1, DO], BF16, tag="bdrow")
            if not flags["bv_zero"] or not flags["bd_zero"]:
                onesrow = wts.tile([1, 128], BF16, tag="onesrow")
            if not flags["ln_identity"]:
                lng = wts.tile([128, DV], F32, tag="lng")
                lnb = wts.tile([128, DV], F32, tag="lnb")

            def load_secondary_weights():
                # late-needed weights: few triggers, gpsimd queue
                dma_split_k(wd8, wd8_d[:], 1, eng=nc.gpsimd)
                if not flags["bv_zero"]:
                    nc.gpsimd.dma_start(bvrow[:], bvrow_d[:])
                if not flags["bd_zero"]:
                    nc.gpsimd.dma_start(bdrow[:], bdrow_d[:])
                if not flags["bv_zero"] or not flags["bd_zero"]:
                    nc.gpsimd.dma_start(onesrow[:], onesrow_d[:])
                if not flags["ln_identity"]:
                    nc.gpsimd.dma_start(lng[:], lng_d[:])
                    nc.gpsimd.dma_start(lnb[:], lnb_d[:])

            def stage_c_open(ctx8):
                # first dense half (heads 0-3 / e-blocks 0,1) for each s-block
                dpss = []
                for sb in range(KB):
                    dps = psAD.tile([128, 512], F32, tag="psAD", name="dps")
                    nc.tensor.matmul(
                        dps[:],
                        ctx8[:, 0:2, bass.ts(sb, 128)],
                        wd8[:, 0:2, :],
                        start=True,
                        stop=False,
                        perf_mode=DR,
                    )
                    dpss.append(dps)
                return dpss

            def stage_c_close(bb, ctx8, xr, dpss):
                last_b = bb == BPC - 1
                for sb in range(KB):
                    dps = dpss[sb]
                    nc.tensor.matmul(
                        dps[:],
                        ctx8[:, 2:4, bass.ts(sb, 128)],
                        wd8[:, 2:4, :],
                        start=False,
                        stop=flags["bd_zero"],
                        perf_mode=DR,
                    )
                    if not flags["bd_zero"]:
                        nc.tensor.matmul(
                            dps[:], onesrow[:], bdrow[:], start=False, stop=True
                        )
                    ysb = ysb_p.tile([128, DO], BF16, tag="ysb")
                    nc.vector.scalar_tensor_tensor(
                        out=ysb[:],
                        in0=dps[:],
                        scalar=1.0 / SCD,
                        in1=xr[:, sb, :],
                        op0=mul,
                        op1=add,
                    )
                    st6 = st_p.tile([128, 6], F32, tag="st6")
                    nc.vector.bn_stats(st6[:], ysb[:])
                    mv = st_p.tile([128, 2], F32, tag="st")
                    nc.vector.bn_aggr(mv[:], st6[:])
                    lnv = st_p.tile([128, 1], F32, tag="st")
                    nc.scalar.activation(
                        lnv[:], mv[:, 1:2], AF.Ln, bias=consts[:, 1:2]
                    )
                    rstd = st_p.tile([128, 1], F32, tag="st")
                    nc.scalar.activation(rstd[:], lnv[:], AF.Exp, scale=-0.5)
                    m2 = st_p.tile([128, 1], F32, tag="st")
                    nc.vector.tensor_scalar(
                        out=m2[:], in0=mv[:, 0:1], scalar1=rstd[:],
                        scalar2=None, op0=mul,
                    )
                    zdst = yout_p.tile([128, DO], BF16, tag="yz")
                    nc.vector.tensor_scalar(
                        out=zdst[:],
                        in0=ysb[:],
                        scalar1=rstd[:],
                        scalar2=m2[:],
                        op0=mul,
                        op1=sub,
                    )
                    if not flags["ln_identity"]:
                        z2 = ysb_p.tile([128, DO], F32, tag="z2")
                        nc.vector.tensor_mul(z2[:], zdst[:], lng[:])
                        zf = yout_p.tile([128, DO], BF16, tag="yzf")
                        nc.vector.tensor_add(zf[:], z2[:], lnb[:])
                        zdst = zf
                    # y out: split per 64-partition half so the last transfer
                    # isn't a 6.8us single-queue tail; 4-way for the last one
                    nsp = 4 if (last_b and sb == KB - 1) else 2
                    pp = 128 // nsp
                    for sp in range(nsp):
                        nc.sync.dma_start(
                            y_d[bb, sb * 128 + sp * pp : sb * 128 + (sp + 1) * pp, :],
                            zdst[sp * pp : (sp + 1) * pp, :],
                        )

            def emit_stage_c(bb, ctx8, xr):
                stage_c_close(bb, ctx8, xr, stage_c_open(ctx8))

            pending = []
            for b in range(BPC):
                # ---- load per-batch activations; q-path data first,
                # chunk-interleaved with its weights on the sync queue
                xtq8 = xt_p.tile([128, KB, S], FP8, tag="xtq")
                xt8 = xt_p.tile([128, KB, S], FP8, tag="xt")
                if b == 0:
                    for c in range(KB):
                        dma_chunk(xtq8, xtq8_d[b], c)
                        dma_chunk(wq8, wq8_d[:], c)
                    for c in range(KB):
                        dma_chunk(xt8, xt8_d[b], c)
                        dma_chunk(wk8, wk8_d[:], c)
                    dma_split_k(wv8, wv8_d[:], 2)
                    nc.sync.dma_start(
                        mixt[:], mixt_d[:].rearrange("(k p) h -> p k h", p=128)
                    )
                    nc.sync.dma_start(consts[:], consts_d[:])
                else:
                    dma_split_k(xtq8, xtq8_d[b])
                    dma_split_k(xt8, xt8_d[b])
                eb = [
                    eb_p.tile([128, S], BF16, tag=f"eb{t}", name="eb")
                    for t in range(KB)
                ]
                for t in range(KB):
                    # first two t-blocks split for earlier availability
                    if t < 2:
                        for hp in range(2):
                            nc.gpsimd.dma_start(
                                eb[t][64 * hp : 64 * hp + 64, :],
                                eb_d[b, t * 128 + 64 * hp : t * 128 + 64 * hp + 64, :],
                            )
                    else:
                        nc.gpsimd.dma_start(eb[t][:], eb_d[b, bass.ts(t, 128), :])
                cb = cb_p.tile([128, KB * H], F32, tag="cb")
                nc.gpsimd.dma_start(cb[:], cb_d[b])
                if b == 0:
                    load_secondary_weights()
                xr = xr_p.tile([128, KB, D], BF16, tag="xr")
                dma_split_k(xr, xr_d[b], eng=nc.gpsimd)

                # ---- q/k/v projections (fp8 DoubleRow), T-major outputs
                qt = [
                    qt_p.tile([128, DK], BF16, tag="qt", name="qt")
                    for _ in range(KB)
                ]
                kt = qkv_p.tile([128, KB, DK], BF16, tag="kt")
                vt = qkv_p.tile([128, KB, DV], BF16, tag="vt")

                for w, src, dst2, dst3 in (
                    (wq8, xtq8, qt, None),
                    (wk8, xt8, None, kt),
                ):
                    # dst[i, s] = sum_d w[d, i] * xT[d, s]   (i in partitions)
                    for i in range(KB):
                        ps = psAD.tile([128, 512], F32, tag="psAD", name="ps")
                        for kp in range(2):
                            nc.tensor.matmul(
                                ps[:],
                                w[:, 2 * kp : 2 * kp + 2, bass.ts(i, 128)],
                                src[:, 2 * kp : 2 * kp + 2, :],
                                start=(kp == 0),
                                stop=(kp == 1),
                                perf_mode=DR,
                            )
                        dst = dst2[i][:] if dst2 is not None else dst3[:, i, :]
                        nc.vector.tensor_copy(dst, ps[:])
                # v[t, j] = sum_d xT[d, t] * Wv.T[d, j] (+ bv)  (t in partitions)
                for i in range(KB):
                    ps = psAD.tile([128, 512], F32, tag="psAD")
                    for kp in range(2):
                        last = kp == 1 and flags["bv_zero"]
                        nc.tensor.matmul(
                            ps[:],
                            xt8[:, 2 * kp : 2 * kp + 2, bass.ts(i, 128)],
                            wv8[:, 2 * kp : 2 * kp + 2, :],
                            start=(kp == 0),
                            stop=last,
                            perf_mode=DR,
                        )
                    if not flags["bv_zero"]:
                        nc.tensor.matmul(
                            ps[:], onesrow[:], bvrow[:], start=False, stop=True
                        )
                    nc.vector.tensor_copy(vt[:, i, :], ps[:])

                # previous batch's output stage goes here so the PE has
                # dense work while this batch's first heads wait on DVE/ACT
                if pending:
                    emit_stage_c(*pending.pop())

                # ---- per-head attention
                ctx8 = ctx_p.tile([128, KB, S], FP8, tag="ctx8")
                for h in range(H):
                    mq = mq_p.tile([128, KB, S], BF16, tag="mq")
                    for k in range(KB):
                        nc.vector.tensor_scalar_mul(
                            mq[:, k, :], qt[k][:], mixt[:, k, h : h + 1]
                        )
                    # stationary [v_h | ones] for the fused ctx+denominator
                    vcat = vc_p.tile([128, KB, 128], BF16, tag="vcat")
                    nc.gpsimd.memset(vcat[:, :, EH:128], 1.0)
                    nc.vector.tensor_copy(
                        vcat[:, :, 0:EH], vt[:, :, h * EH : (h + 1) * EH]
                    )
                    pts = []
                    for t in range(KB):
                        a = 0 if t == 0 else (t * 128 - 2) // 32 * 32
                        sps = psS.tile([128, S], F32, tag="psS", name="sps")
                        for k in range(KB):
                            nc.tensor.matmul(
                                sps[:, a:],
                                kt[:, k, bass.ts(t, 128)],
                                mq[:, k, a:],
                                start=(k == 0),
                                stop=(k == KB - 1),
                            )
                        ptx = ptx_p.tile([128, S], BF16, tag="ptx")
                        nc.scalar.activation(
                            ptx[:, a:],
                            sps[:, a:],
                            AF.Exp,
                            bias=cb[:, H * t + h : H * t + h + 1],
                            scale=EXP_SCALE,
                        )
                        pt = pt_p.tile([128, S], BF16, tag="pt")
                        e = nc.vector if t < 2 else nc.gpsimd
                        e.tensor_mul(pt[:, a:], ptx[:, a:], eb[t][:, a:])
                        pts.append((pt, a))
                    # fused ctx+denominator: stationary [v_h | ones] via a
                    # strided slice (slots h and 8 of vt) -> rows 0:64 ctx,
                    # rows 64:128 denominator replicated
                    cu = psCU.tile([128, S], F32, tag="psCU")
                    for t in range(KB):
                        pt, a = pts[t]
                        nc.tensor.matmul(
                            cu[:, a:],
                            vcat[:, t, :],
                            pt[:, a:],
                            start=(t == 0),
                            stop=(t == KB - 1),
                        )
                    rsln = rs_p.tile([64, S], F32, tag="rsln")
                    nc.scalar.activation(rsln[:], cu[64:128, :], AF.Ln)
                    rsum = rs_p.tile([64, S], F32, tag="rs")
                    # 2/den: ln2 bias folds the 64/32 ctx8 upscale
                    nc.scalar.activation(
                        rsum[:], rsln[:], AF.Exp, scale=-1.0,
                        bias=consts[64:128, 0:1],
                    )
                    nc.vector.tensor_mul(
                        ctx8[64 * (h % 2) : 64 * (h % 2) + 64, h // 2, :],
                        cu[0:64, :],
                        rsum[:],
                    )
                    if b == BPC - 1 and h == 3:
                        # last batch: open the dense stage early so its first
                        # half overlaps heads 4-7 instead of trailing them
                        dpss_last = stage_c_open(ctx8)
                if b == BPC - 1:
                    stage_c_close(b, ctx8, xr, dpss_last)
                else:
                    pending.append((b, ctx8, xr))

    _split_multi_waits(nc)
    return nc


# ------------------------------------------------------------------- driver
def _run(inputs, trace=False, trace_kwargs=None):
    in_maps, flags = _prepare(inputs)
    nc = build_nc(flags)
    res = run_bass_kernel_spmd(
        nc,
        in_maps,
        list(range(N_CORES)),
        trace=trace,
        **(trace_kwargs or {}),
    )
    PERM = np.concatenate([np.arange(1, S), [0]])
    out = np.empty((B, S, DO), np.float32)
    for c in range(N_CORES):
        out[c * BPC : (c + 1) * BPC][:, PERM, :] = np.asarray(
            res.results[c]["y"]
        ).astype(np.float32)
    return out, res


def kernel(**inputs) -> np.ndarray:
    out, _ = _run(inputs, trace=False)
    return out
